# revision 12
# baseline (speedup 1.0000x reference)
"""Trainium2 Bass kernel for nn_ExpertGroup (moe_routing).

Contract: kernel(**inputs) takes FULL unsharded numpy inputs and returns the
FULL [2, 2048, 1024] fp32 output. Internally shards B*S=4096 tokens across
8 NeuronCores (512 tokens/core; cores 0-3 own batch 0, cores 4-7 batch 1),
replicates the small weights, and exchanges the per-batch adapter tensors
(adapt_in / adapt_out, [S,128] each) with two intra-group AllGathers.

All matmuls run in bf16 with fp32 PSUM accumulation. Activations are laid out
feature-major ([feature, token]) so every matmul contracts over partitions.

v3 schedule (vs v2 at 182us). Trace findings v2: 11us dead lead-in (DMA issue
cost ~0.6us each x 41 issues), 4us PE gap at 31us (AIT xbar transposes jam
the scalar queue ahead of the up/gate silu evacuations), AO-LN tail takes
29.5us (DRAM-bounce reciprocal dance) so AG2 triggers at 131.5us and the PE
idles 9.4us + re-warms cold. Changes:
  - DMA consolidation: 16 issues instead of 41 (xT in 4, consts packed into
    pk1/ewanc, ug in 6, wfin in 2, own SBUF slot - no WAR streaming games)
  - pre-GEMM k-outer so the 4 psum groups stream against the xT DMA
  - AIT transposes moved to the gpsimd queue (idle until the AG1 trigger)
  - AO-LN tail: stats via ones-matmul (as v2) but rstd via
    reciprocal_approx_fast on the [1,T] row (no DRAM bounce, no
    re-partition), broadcast back via two K=1 ones-row matmuls into PSUM
  - act-table hoists: dummy Sqrt/Silu activations pinned to AOf/AOT
    readiness so the real AO sqrt and the bmm1 silus hit a warm table
  - mixedT contribution folded into the final_down accumulation (ready at
    ~65us), so final_close is one matmul per dc
  - facc in bf16, evacuations alternate scalar/vector
"""

import sys

sys.path.insert(0, "/opt/trn_rl_repo")

import ml_dtypes
import numpy as np

import concourse.bass as bass
import concourse.mybir as mybir
import concourse.tile as tile
from concourse import masks
from concourse import bacc
from concourse.bass_utils import run_bass_kernel_spmd

BF16 = mybir.dt.bfloat16
F32 = mybir.dt.float32

B, S, D, E = 2, 2048, 1024, 8
H = 2 * D          # 2048
A = H // 16        # 128
N = B * S          # 4096
NCORES = 8
T = N // NCORES    # 512 tokens per core
GROUP = 4          # cores per batch
SC = T // 128      # 4 s-chunks per core
DC = D // 128      # 8 d-chunks (output features)
HC = H // 128      # 16 h-chunks
KD = D // 128      # 8 k-chunks over D
TC_FULL = S // 128  # 16 token-chunks per batch
EPS = 1e-5

_CACHE = {}


def _build():
    nc = bacc.Bacc(None, num_devices=NCORES)

    # ---- kernel I/O (per-core; weights pre-packed to SBUF layout on host) ----
    xT_d = nc.dram_tensor("xT", [128, KD, T], BF16, kind="ExternalInput")
    ug_d = nc.dram_tensor("ug_hc", [128, HC, KD * 256], BF16, kind="ExternalInput")
    pre_d = nc.dram_tensor("pre_wT", [128, KD, A], BF16, kind="ExternalInput")
    post_d = nc.dram_tensor("post_wT", [128, A, HC], BF16, kind="ExternalInput")
    pk1_d = nc.dram_tensor("pk1", [128, SC * A + E * A], BF16, kind="ExternalInput")
    ewanc_d = nc.dram_tensor("ewanc", [128, SC * E + 2], F32, kind="ExternalInput")
    wfin_d = nc.dram_tensor("wfin", [128, DC, (HC + 2) * 128], BF16, kind="ExternalInput")
    angb_d = nc.dram_tensor("angb", [2, A], F32, kind="ExternalInput")
    ag_d = nc.dram_tensor("ag_row", [1, A * E], BF16, kind="ExternalInput")
    out_d = nc.dram_tensor("out", [D, T], BF16, kind="ExternalOutput")

    # ---- collective bounce buffers (internal DRAM) ----
    ag1_in = nc.dram_tensor("ag1_in", [T, A], BF16)
    ag1_out = nc.dram_tensor("ag1_out", [S, A], BF16)
    ag2_in = nc.dram_tensor("ag2_in", [A, T], BF16)
    ag2_out = nc.dram_tensor("ag2_out", [GROUP * A, T], BF16)
    RG = [[0, 1, 2, 3], [4, 5, 6, 7]]

    with tile.TileContext(nc) as tc:
        with (
            tc.tile_pool(name="consts", bufs=1) as consts,
            tc.tile_pool(name="wpool", bufs=1) as wpool,
            tc.tile_pool(name="acts", bufs=1) as acts,
            tc.tile_pool(name="work", bufs=4) as work,
            tc.tile_pool(name="work2", bufs=2) as work2,
            tc.tile_pool(name="wtp", bufs=3) as wtp,
            tc.tile_pool(name="workbig", bufs=1) as workbig,
            tc.tile_pool(name="aoln", bufs=1) as aoln,
            tc.tile_pool(name="evac", bufs=4) as evac,
            tc.tile_pool(name="ps_big", bufs=4, space="PSUM") as ps_big,
            tc.tile_pool(name="ps_po", bufs=1, space="PSUM") as ps_po,
            tc.tile_pool(name="ps_out", bufs=2, space="PSUM") as ps_out,
            tc.tile_pool(name="ps_acc", bufs=1, space="PSUM") as ps_acc,
        ):
            # ---------- DMA issue order (sync queue): the pre-matmul chain
            # first, then up/gate weights interleaved with packed consts.
            # Each dma_start costs ~0.6us of queue issue time, so consolidate.
            # scalar queue carries the small early loads so they overlap the
            # sync queue's xT/ug stream on the SDMA rings
            pre_w = wpool.tile([128, KD, A], BF16)
            nc.scalar.dma_start(out=pre_w, in_=pre_d[:])
            xT = wpool.tile([128, KD, T], BF16)
            for k in range(KD):
                eng = nc.sync if k % 2 == 0 else nc.scalar
                eng.dma_start(out=xT[:, k:k + 1, :], in_=xT_d[:, k:k + 1, :])
            ug_w = wpool.tile([128, HC, KD, 256], BF16, tag="bigw")
            nc.sync.dma_start(out=ug_w[:, 0:2], in_=ug_d[:, 0:2, :])
            gB = consts.tile([128, A], F32)   # an_g broadcast across partitions
            bB = consts.tile([128, A], F32)   # an_b broadcast
            nc.scalar.dma_start(
                out=gB,
                in_=bass.AP(tensor=angb_d, offset=0, ap=[[0, 128], [1, A]]),
            )
            nc.scalar.dma_start(
                out=bB,
                in_=bass.AP(tensor=angb_d, offset=A, ap=[[0, 128], [1, A]]),
            )
            angb_sb = consts.tile([2, A], F32)   # raw an_g / an_b rows
            nc.scalar.dma_start(out=angb_sb, in_=angb_d[:])
            ewanc = consts.tile([128, SC * E + 2], F32)    # ew | ancol
            nc.scalar.dma_start(out=ewanc, in_=ewanc_d[:])
            agB = consts.tile([128, E, A], BF16)  # adapter_g (e-major) bcast
            nc.scalar.dma_start(
                out=agB,
                in_=bass.AP(tensor=ag_d, offset=0, ap=[[0, 128], [A, E], [1, A]]),
            )
            pk1 = wpool.tile([128, SC * A + E * A], BF16)  # bias_mix | adapter_wT
            nc.sync.dma_start(out=pk1, in_=pk1_d[:])
            post_w = wpool.tile([128, A, HC], BF16)
            nc.sync.dma_start(out=post_w, in_=post_d[:])
            nc.sync.dma_start(out=ug_w[:, 2:4], in_=ug_d[:, 2:4, :])
            for i in range(1, 4):
                nc.sync.dma_start(
                    out=ug_w[:, 4 * i:4 * i + 4], in_=ug_d[:, 4 * i:4 * i + 4, :]
                )
            wfin = wpool.tile([128, DC, HC + 2, 128], BF16, tag="bigw")

            # ---------- memset constants (vector queue) ----------
            eps_t = consts.tile([128, 1], F32)
            nc.vector.memset(eps_t, EPS)
            ones_col = consts.tile([128, 1], BF16)
            nc.vector.memset(ones_col, 1.0)
            ones_row = consts.tile([1, 128], BF16)
            nc.vector.memset(ones_row, 1.0)
            ident = consts.tile([128, 128], BF16)
            masks.make_identity(nc, ident[:])

            # persistent activations
            AI_tok = acts.tile([128, SC, A], BF16)    # adapt_in, token-major
            AIT = acts.tile([128, T], BF16)           # adapt_in, feature-major
            hid = acts.tile([128, HC, T], BF16)       # hidden, feature-major
            AOTfull = acts.tile([128, GROUP, T], BF16)   # gathered AO feat-major
            AOT = acts.tile([128, T], BF16)           # local AO, feature-major
            adaptT = acts.tile([128, T], BF16)        # adapt, feature-major
            mixedT = acts.tile([128, T], BF16)        # mixed, feature-major
            mix_tok = acts.tile([128, SC, A], BF16)   # mixed, token-major
            facc = acts.tile([128, DC, T], BF16)      # down+mixed partial acc

            def layernorm_to(ps, dst):
                """LN over free dim (A=128) of psum tile [128, A]; write dst bf16."""
                st = work.tile([128, 6], F32, tag="lnst")
                nc.vector.bn_stats(out=st, in_=ps)
                mv = work.tile([128, 2], F32, tag="lnmv")
                nc.vector.bn_aggr(out=mv, in_=st)
                sd = work.tile([128, 1], F32, tag="lnsd")
                nc.scalar.activation(
                    out=sd, in_=mv[:, 1:2], func=mybir.ActivationFunctionType.Sqrt,
                    bias=eps_t, scale=1.0,
                )
                r = work.tile([128, 1], F32, tag="lnr")
                nc.vector.reciprocal(out=r, in_=sd)
                z = work.tile([128, A], F32, tag="lnz")
                nc.vector.tensor_scalar(
                    out=z, in0=ps, scalar1=mv[:, 0:1], scalar2=r,
                    op0=mybir.AluOpType.subtract, op1=mybir.AluOpType.mult,
                )
                zg = work.tile([128, A], F32, tag="lnzg")
                nc.vector.tensor_tensor(out=zg, in0=z, in1=gB, op=mybir.AluOpType.mult)
                nc.vector.tensor_tensor(out=dst, in0=zg, in1=bB, op=mybir.AluOpType.add)

            # ---------- adapt_in = LN(x @ pre_w.T): k-outer so the 4 psum
            # groups stream against the xT DMA chunks as they arrive.
            pre_ps = [
                ps_big.tile([128, A], F32, tag="mm", name=f"pre_ps{sc}")
                for sc in range(SC)
            ]
            for k in range(KD):
                for sc in range(SC):
                    nc.tensor.matmul(
                        pre_ps[sc], xT[:, k, sc * 128:(sc + 1) * 128], pre_w[:, k, :],
                        start=(k == 0), stop=(k == KD - 1),
                    )
            for sc in range(SC):
                layernorm_to(pre_ps[sc], AI_tok[:, sc, :])

            # AIT transposes via the PE (ring-independent: a pending
            # collective freezes DMA rings, so transposes must not ride them).
            for sc in range(SC):
                tp = ps_out.tile([128, 128], BF16, tag="fout")
                nc.tensor.transpose(tp, AI_tok[:, sc, :], ident[:])
                if sc % 2 == 0:
                    nc.scalar.copy(out=AIT[:, sc * 128:(sc + 1) * 128], in_=tp)
                else:
                    nc.vector.tensor_copy(
                        out=AIT[:, sc * 128:(sc + 1) * 128], in_=tp
                    )
            # wfin, then AG1 staging LAST on the sync ring: ring FIFO puts the
            # staging transfer after every weight transfer, so the pending
            # collective can never starve the weight stream (v3 lost 17us to
            # exactly that; v4 lost 24us to the transposes queued behind it).
            nc.sync.dma_start(out=wfin[:, 0:4], in_=wfin_d[:, 0:4, :])
            nc.sync.dma_start(out=wfin[:, 4:8], in_=wfin_d[:, 4:8, :])
            nc.sync.dma_start(
                out=ag1_in[:].rearrange("(sc p) a -> p sc a", p=128), in_=AI_tok
            )
            nc.gpsimd.collective_compute(
                "AllGather", mybir.AluOpType.bypass, replica_groups=RG,
                ins=[ag1_in[:]], outs=[ag1_out[:]],
            )
            AIfull = acts.tile([128, TC_FULL, A], BF16)   # gathered AI token-major
            nc.gpsimd.dma_start(
                out=AIfull, in_=ag1_out[:].rearrange("(k p) a -> p k a", p=128)
            )

            # ---------- expert path, part 1 (injected into the up/gate loop) ---
            hsb = acts.tile([128, SC, E * A], BF16)   # h, token-major, e-outer
            mv8s = work.tile([128, SC, E, 2], F32, tag="mv8s", bufs=1)

            def expert_mm(sc):
                hp0 = ps_out.tile([128, 512], F32, tag="fout")
                hp1 = ps_out.tile([128, 512], F32, tag="fout")
                sl = AIT[:, sc * 128:(sc + 1) * 128]
                nc.tensor.matmul(hp0, sl, pk1[:, SC * A:SC * A + 512],
                                 start=True, stop=True)
                nc.tensor.matmul(hp1, sl, pk1[:, SC * A + 512:SC * A + 1024],
                                 start=True, stop=True)
                st8 = work.tile([128, E, 6], F32, tag="st8")
                hps = [hp0, hp0, hp0, hp0, hp1, hp1, hp1, hp1]
                for e in range(E):
                    nc.vector.bn_stats(
                        out=st8[:, e, :], in_=hps[e][:, (e % 4) * A:(e % 4 + 1) * A]
                    )
                for e in range(E):
                    nc.vector.bn_aggr(out=mv8s[:, sc, e, :], in_=st8[:, e, :])
                nc.scalar.copy(out=hsb[:, sc, 0:512], in_=hp0)
                nc.scalar.copy(out=hsb[:, sc, 512:1024], in_=hp1)

            def expert_ln(sc, r8all):
                rw8 = work.tile([128, E], F32, tag="rw8")
                nc.vector.tensor_tensor(
                    out=rw8, in0=r8all[:, sc, :],
                    in1=ewanc[:, sc * E:(sc + 1) * E],
                    op=mybir.AluOpType.mult,
                )
                nmrw = work.tile([128, E], F32, tag="nmrw")
                nc.vector.tensor_tensor(
                    out=nmrw, in0=mv8s[:, sc, :, 0], in1=rw8,
                    op=mybir.AluOpType.mult,
                )
                nc.vector.tensor_scalar(
                    out=nmrw, in0=nmrw, scalar1=-1.0, scalar2=None,
                    op0=mybir.AluOpType.mult,
                )
                zt = workbig.tile([128, E, A], BF16, tag="zt")
                for e in range(E):
                    nc.vector.tensor_scalar(
                        out=zt[:, e, :], in0=hsb[:, sc, e * A:(e + 1) * A],
                        scalar1=rw8[:, e:e + 1], scalar2=nmrw[:, e:e + 1],
                        op0=mybir.AluOpType.mult, op1=mybir.AluOpType.add,
                    )
                zg = workbig.tile([128, E, A], BF16, tag="ztg")
                nc.vector.tensor_tensor(
                    out=zg, in0=zt, in1=agB, op=mybir.AluOpType.mult
                )
                t1 = workbig.tile([128, 4, A], BF16, tag="sum1")
                nc.vector.tensor_tensor(
                    out=t1, in0=zg[:, 0:4, :], in1=zg[:, 4:8, :],
                    op=mybir.AluOpType.add,
                )
                t2 = work.tile([128, 2, A], BF16, tag="sum2")
                nc.vector.tensor_tensor(
                    out=t2, in0=t1[:, 0:2, :], in1=t1[:, 2:4, :],
                    op=mybir.AluOpType.add,
                )
                mx = work.tile([128, A], BF16, tag="mx")
                nc.vector.tensor_tensor(
                    out=mx, in0=t2[:, 0, :], in1=t2[:, 1, :], op=mybir.AluOpType.add
                )
                nc.vector.tensor_tensor(
                    out=mix_tok[:, sc, :], in0=mx,
                    in1=pk1[:, sc * A:(sc + 1) * A],
                    op=mybir.AluOpType.add,
                )

            # ---------- hidden = silu(x@gate.T) * (x@up.T), feature-major -------
            po_ps = ps_po.tile([128, T], F32, tag="po")

            def post_step(k):
                nc.tensor.matmul(
                    po_ps, post_w[:, :, k], hid[:, k, :],
                    start=(k == 0), stop=(k == HC - 1),
                )

            for hc in range(HC):
                up_ps = ps_big.tile([128, T], F32, tag="mm")
                gt_ps = ps_big.tile([128, T], F32, tag="mm")
                for k in range(KD):
                    nc.tensor.matmul(
                        up_ps, ug_w[:, hc, k, 0:128], xT[:, k, :],
                        start=(k == 0), stop=(k == KD - 1),
                    )
                for k in range(KD):
                    nc.tensor.matmul(
                        gt_ps, ug_w[:, hc, k, 128:256], xT[:, k, :],
                        start=(k == 0), stop=(k == KD - 1),
                    )
                sg = work2.tile([128, T], BF16, tag="sg")
                nc.scalar.activation(
                    out=sg, in_=gt_ps, func=mybir.ActivationFunctionType.Silu
                )
                nc.vector.tensor_tensor(
                    out=hid[:, hc, :], in0=sg, in1=up_ps, op=mybir.AluOpType.mult
                )
                if hc > 0:
                    post_step(hc - 1)
                if 3 <= hc <= 6:
                    expert_mm(hc - 3)
            post_step(HC - 1)

            # ---------- expert path, part 2 (runs mid-up/gate off mv8s) ------
            sd8all = work.tile([128, SC, E], F32, tag="sd8all", bufs=1)
            nc.scalar.activation(
                out=sd8all, in_=mv8s[:, :, :, 1],
                func=mybir.ActivationFunctionType.Sqrt, bias=eps_t, scale=1.0,
            )
            r8all = work.tile([128, SC, E], F32, tag="r8all", bufs=1)
            nc.vector.reciprocal(out=r8all, in_=sd8all)
            for sc in range(SC):
                expert_ln(sc, r8all)
            # mixed -> feature-major via PE transposes (ring-independent)
            for sc in range(SC):
                tp = ps_out.tile([128, 128], BF16, tag="fout")
                nc.tensor.transpose(tp, mix_tok[:, sc, :], ident[:])
                if sc % 2 == 0:
                    nc.scalar.copy(out=mixedT[:, sc * 128:(sc + 1) * 128], in_=tp)
                else:
                    nc.vector.tensor_copy(
                        out=mixedT[:, sc * 128:(sc + 1) * 128], in_=tp
                    )

            # ---------- adapt_out LN, feature-major ----------
            # stats via PE ones-matmuls; rstd via reciprocal_approx_fast on the
            # [1,T] row (1-lane DVE but only ~0.7us); broadcast back over
            # partitions with two K=1 ones-row matmuls into PSUM.
            AOf = acts.tile([128, T], BF16)
            nc.scalar.copy(out=AOf, in_=po_ps)
            # dummy Sqrt pinned to AOf: forces the act-table switch now so the
            # real sqrt below doesn't pay the 1.3us load on the AG2 path
            tdum = aoln.tile([1, 1], F32)
            nc.scalar.activation(
                out=tdum, in_=AOf[0:1, 0:1], func=mybir.ActivationFunctionType.Sqrt,
                bias=eps_t[0:1], scale=1.0,
            )
            sqf = aoln.tile([128, T], BF16)
            nc.vector.tensor_tensor(out=sqf, in0=AOf, in1=AOf, op=mybir.AluOpType.mult)
            s0 = ps_po.tile([1, T], F32, tag="po")
            nc.tensor.matmul(s0, ones_col, AOf, start=True, stop=True)
            s1 = ps_acc.tile([1, T], F32, tag="adps")
            nc.tensor.matmul(s1, ones_col, sqf, start=True, stop=True)
            # stats split across scalar (mean, mean^2) and vector (E[x^2],
            # var, rstd, mean*rstd); an_g is folded into the K=1 broadcast
            # matmuls (lhsT = raw an_g row), so the token-space transform is
            # two fused vector ops.
            mean_b = aoln.tile([1, T], F32)
            nc.scalar.activation(
                out=mean_b, in_=s0, func=mybir.ActivationFunctionType.Copy,
                scale=1.0 / A,
            )
            e2 = aoln.tile([1, T], F32)
            nc.vector.tensor_scalar(
                out=e2, in0=s1, scalar1=1.0 / A, scalar2=None,
                op0=mybir.AluOpType.mult,
            )
            m2 = aoln.tile([1, T], F32)
            nc.scalar.square(out=m2, in_=mean_b)
            varf = aoln.tile([1, T], F32)
            nc.vector.tensor_tensor(out=varf, in0=e2, in1=m2,
                                    op=mybir.AluOpType.subtract)
            sdf = aoln.tile([1, T], F32)
            nc.scalar.activation(
                out=sdf, in_=varf, func=mybir.ActivationFunctionType.Sqrt,
                bias=eps_t[0:1], scale=1.0,
            )
            r32 = aoln.tile([1, T], F32)
            nc.vector.reciprocal_approx_fast(out=r32, in_=sdf)
            mr32 = aoln.tile([1, T], F32)   # mean * rstd (subtracted later)
            nc.vector.tensor_tensor(out=mr32, in0=mean_b, in1=r32,
                                    op=mybir.AluOpType.mult)
            # half of fd(0) fills the PE while the rstd row chain runs
            fd0_ps = ps_out.tile([128, T], F32, tag="fout", name="fd0_ps")
            for k in range(8):
                nc.tensor.matmul(
                    fd0_ps, wfin[:, 0, k, :], hid[:, k, :],
                    start=(k == 0), stop=False,
                )
            grB_ps = ps_big.tile([128, T], F32, tag="mm", name="grB_ps")
            nc.tensor.matmul(grB_ps, angb_sb[0:1, :], r32, start=True, stop=True)
            gmrB_ps = ps_big.tile([128, T], F32, tag="mm", name="gmrB_ps")
            nc.tensor.matmul(gmrB_ps, angb_sb[0:1, :], mr32, start=True, stop=True)
            z1 = aoln.tile([128, T], BF16)
            nc.vector.tensor_tensor(out=z1, in0=AOf, in1=grB_ps,
                                    op=mybir.AluOpType.mult)
            nc.vector.scalar_tensor_tensor(
                out=AOT, in0=z1, scalar=ewanc[:, SC * E + 1:SC * E + 2],
                in1=gmrB_ps, op0=mybir.AluOpType.add, op1=mybir.AluOpType.subtract,
            )
            # stage + trigger AG2
            nc.scalar.dma_start(out=ag2_in[:], in_=AOT)
            nc.gpsimd.collective_compute(
                "AllGather", mybir.AluOpType.bypass, replica_groups=RG,
                ins=[ag2_in[:]], outs=[ag2_out[:]],
            )
            # dummy Silu pinned to AOT: warm the Silu table for the bmm1 silus
            tdum2 = aoln.tile([1, 1], BF16)
            nc.scalar.activation(
                out=tdum2, in_=AOT[0:1, 0:1], func=mybir.ActivationFunctionType.Silu,
            )
            for c in range(GROUP):
                nc.gpsimd.dma_start(
                    out=AOTfull[:, c, :], in_=ag2_out[c * A:(c + 1) * A, :]
                )
            AOTf = AOTfull.rearrange("a c t -> a (c t)")

            # ---------- final_down (down contribution only; mixed+adapt land
            # in final_close so fd never waits on the expert path) ----------
            def final_down(dc):
                if dc == 0:
                    op = fd0_ps
                    ks = range(8, HC)
                else:
                    op = ps_out.tile([128, T], F32, tag="fout")
                    ks = range(HC)
                for k in ks:
                    nc.tensor.matmul(
                        op, wfin[:, dc, k, :], hid[:, k, :],
                        start=(k == 0), stop=(k == HC - 1),
                    )
                if dc % 2 == 0:
                    nc.scalar.copy(out=facc[:, dc, :], in_=op)
                else:
                    nc.vector.tensor_copy(out=facc[:, dc, :], in_=op)

            for dc in range(DC):
                final_down(dc)

            # ---------- w = silu(clip(AI_loc @ AO_full.T)); adapt = w.T chain ---
            ad_ps = ps_acc.tile([128, T], F32, tag="adps")
            wts_buf = {}

            def bmm1_step(j):
                w_ps = ps_big.tile([128, T], F32, tag="mm")
                nc.tensor.matmul(
                    w_ps, AOTf[:, j * 128:(j + 1) * 128], AIT, start=True, stop=True
                )
                wc = work2.tile([128, T], BF16, tag="wc")
                nc.vector.tensor_scalar(
                    out=wc, in0=w_ps, scalar1=-5.0, scalar2=5.0,
                    op0=mybir.AluOpType.max, op1=mybir.AluOpType.min,
                )
                wt = wtp.tile([128, T], BF16, tag="wts")
                nc.scalar.activation(
                    out=wt, in_=wc, func=mybir.ActivationFunctionType.Silu
                )
                wts_buf[j] = wt

            # depth-2 software pipeline: bmm2(j) trails bmm1(j+2) so the PE
            # never waits on the clip/silu stages
            for j in range(TC_FULL):
                bmm1_step(j)
                if j >= 2:
                    nc.tensor.matmul(
                        ad_ps, AIfull[:, j - 2, :], wts_buf.pop(j - 2),
                        start=(j - 2 == 0), stop=False,
                    )
            for j in (TC_FULL - 2, TC_FULL - 1):
                nc.tensor.matmul(
                    ad_ps, AIfull[:, j, :], wts_buf.pop(j),
                    start=False, stop=(j == TC_FULL - 1),
                )
            nc.scalar.copy(out=adaptT, in_=ad_ps)

            # ---------- finish output ----------
            def final_close(dc):
                op = ps_big.tile([128, T], F32, tag="mm")
                nc.tensor.matmul(
                    op, wfin[:, dc, HC, :], adaptT, start=True, stop=False,
                )
                nc.tensor.matmul(
                    op, wfin[:, dc, HC + 1, :], mixedT, start=False, stop=True,
                )
                ob = evac.tile([128, T], BF16, tag="ob")
                nc.vector.tensor_tensor(
                    out=ob, in0=facc[:, dc, :], in1=op, op=mybir.AluOpType.add
                )
                nc.sync.dma_start(out=out_d[dc * 128:(dc + 1) * 128, :], in_=ob)

            for dc in range(DC):
                final_close(dc)

    nc.compile()
    return nc


def kernel(
    x, expert_weights, up_w, gate_w, down_w, pre_w, post_w, an_g, an_b,
    adapt_proj_w, adapter_w, adapter_g, adapter_b, expert_proj_w, output_proj_w,
):
    x = np.asarray(x, np.float32)
    expert_weights = np.asarray(expert_weights, np.float32)
    bf = ml_dtypes.bfloat16

    if "nc" not in _CACHE:
        _CACHE["nc"] = _build()
    nc = _CACHE["nc"]

    def pack(w, kc):
        # [kc*128, F] -> [128, kc, F] (partition-major SBUF layout)
        f = w.shape[1]
        return np.ascontiguousarray(
            w.reshape(kc, 128, f).transpose(1, 0, 2)
        ).astype(bf)

    ug_wT = np.concatenate(
        [np.asarray(up_w, np.float32), np.asarray(gate_w, np.float32)], axis=0
    ).T                                                        # [D, 2H]
    # hc-major pack: [128, HC, KD*256]; per hc: KD chunks of (up 128 | gate 128)
    up_part = ug_wT[:, :H].reshape(KD, 128, HC, 128)
    gt_part = ug_wT[:, H:].reshape(KD, 128, HC, 128)
    ug_hc = np.ascontiguousarray(
        np.stack([up_part, gt_part], axis=3)        # [k, p, hc, 2, 128]
        .transpose(1, 2, 0, 3, 4)                   # [p, hc, k, 2, 128]
        .reshape(128, HC, KD * 256)
    ).astype(bf)
    pre_wT = np.asarray(pre_w, np.float32).T                   # [D, A]
    post_pack = np.ascontiguousarray(
        np.asarray(post_w, np.float32).T.reshape(HC, 128, A).transpose(1, 2, 0)
    ).astype(bf)                                               # [128, A, HC]
    adapter_wT = (
        np.asarray(adapter_w, np.float32).transpose(2, 0, 1).reshape(A, E * A)
    ).astype(bf)                                               # [A, E*A] (e-major)
    down_w = np.asarray(down_w, np.float32)
    w_da = 0.1 * (down_w @ np.asarray(adapt_proj_w, np.float32))       # [D, A]
    w_mo = np.asarray(output_proj_w, np.float32) @ np.asarray(
        expert_proj_w, np.float32
    )                                                                   # [D, A]
    wfin = np.concatenate([down_w.T, w_da.T, w_mo.T], axis=0)  # [2304, D]
    angb = np.stack(
        [np.asarray(an_g, np.float32), np.asarray(an_b, np.float32)], axis=0
    )                                                                   # [2, A]
    ancol = np.ascontiguousarray(angb.T)                                # [A, 2]
    ag_row = np.asarray(adapter_g, np.float32).reshape(1, A * E).astype(bf)  # e-major
    bias_mix = (expert_weights @ np.asarray(adapter_b, np.float32)).astype(bf)

    xf = x.reshape(N, D)
    shared = {
        "ug_hc": ug_hc, "pre_wT": pack(pre_wT, KD),
        "post_wT": post_pack,
        "wfin": np.ascontiguousarray(
            wfin.reshape(HC + 2, 128, DC, 128).transpose(1, 2, 0, 3)
            .reshape(128, DC, (HC + 2) * 128)
        ).astype(bf), "angb": angb,
        "ag_row": ag_row,
    }
    in_maps = []
    for c in range(NCORES):
        sl = slice(c * T, (c + 1) * T)
        ewc = np.ascontiguousarray(expert_weights[sl]).reshape(SC, 128, E)
        ew_c = np.ascontiguousarray(ewc.transpose(1, 0, 2)).reshape(128, SC * E)
        bmc = np.ascontiguousarray(bias_mix[sl]).reshape(SC, 128, A)
        bm_c = np.ascontiguousarray(bmc.transpose(1, 0, 2)).reshape(128, SC * A)
        pk1_c = np.concatenate(
            [bm_c, np.broadcast_to(adapter_wT, (128, E * A)).astype(bf)], axis=1
        )
        ewanc_c = np.concatenate(
            [ew_c.astype(np.float32),
             np.broadcast_to(ancol.reshape(A, 2), (128, 2)).astype(np.float32)],
            axis=1,
        )
        in_maps.append(
            dict(
                shared,
                xT=pack(np.ascontiguousarray(xf[sl].T), KD),
                pk1=np.ascontiguousarray(pk1_c),
                ewanc=np.ascontiguousarray(ewanc_c),
            )
        )

    try:
        res = run_bass_kernel_spmd(nc, in_maps, list(range(NCORES))).results
    except Exception:
        # axon workers occasionally hang up; one retry on a fresh dispatch
        import time

        time.sleep(10)
        res = run_bass_kernel_spmd(nc, in_maps, list(range(NCORES))).results
    out = np.empty((N, D), np.float32)
    for c in range(NCORES):
        out[c * T:(c + 1) * T] = np.asarray(res[c]["out"], np.float32).T
    return out.reshape(B, S, D)


# revision 13
# speedup vs baseline: 1.0320x; 1.0320x over previous
"""Trainium2 Bass kernel for nn_ExpertGroup (moe_routing).

Contract: kernel(**inputs) takes FULL unsharded numpy inputs and returns the
FULL [2, 2048, 1024] fp32 output. Internally shards B*S=4096 tokens across
8 NeuronCores (512 tokens/core; cores 0-3 own batch 0, cores 4-7 batch 1),
replicates the small weights, and exchanges the per-batch adapter tensors
(adapt_in / adapt_out, [S,128] each) with two intra-group AllGathers.

All matmuls run in bf16 with fp32 PSUM accumulation. Activations are laid out
feature-major ([feature, token]) so every matmul contracts over partitions.

v3 schedule (vs v2 at 182us). Trace findings v2: 11us dead lead-in (DMA issue
cost ~0.6us each x 41 issues), 4us PE gap at 31us (AIT xbar transposes jam
the scalar queue ahead of the up/gate silu evacuations), AO-LN tail takes
29.5us (DRAM-bounce reciprocal dance) so AG2 triggers at 131.5us and the PE
idles 9.4us + re-warms cold. Changes:
  - DMA consolidation: 16 issues instead of 41 (xT in 4, consts packed into
    pk1/ewanc, ug in 6, wfin in 2, own SBUF slot - no WAR streaming games)
  - pre-GEMM k-outer so the 4 psum groups stream against the xT DMA
  - AIT transposes moved to the gpsimd queue (idle until the AG1 trigger)
  - AO-LN tail: stats via ones-matmul (as v2) but rstd via
    reciprocal_approx_fast on the [1,T] row (no DRAM bounce, no
    re-partition), broadcast back via two K=1 ones-row matmuls into PSUM
  - act-table hoists: dummy Sqrt/Silu activations pinned to AOf/AOT
    readiness so the real AO sqrt and the bmm1 silus hit a warm table
  - mixedT contribution folded into the final_down accumulation (ready at
    ~65us), so final_close is one matmul per dc
  - facc in bf16, evacuations alternate scalar/vector
"""

import sys

sys.path.insert(0, "/opt/trn_rl_repo")

import ml_dtypes
import numpy as np

import concourse.bass as bass
import concourse.mybir as mybir
import concourse.tile as tile
from concourse import masks
from concourse import bacc
from concourse.bass_utils import run_bass_kernel_spmd

BF16 = mybir.dt.bfloat16
F32 = mybir.dt.float32

B, S, D, E = 2, 2048, 1024, 8
H = 2 * D          # 2048
A = H // 16        # 128
N = B * S          # 4096
NCORES = 8
T = N // NCORES    # 512 tokens per core
GROUP = 4          # cores per batch
SC = T // 128      # 4 s-chunks per core
DC = D // 128      # 8 d-chunks (output features)
HC = H // 128      # 16 h-chunks
KD = D // 128      # 8 k-chunks over D
TC_FULL = S // 128  # 16 token-chunks per batch
EPS = 1e-5

_CACHE = {}


def _build():
    nc = bacc.Bacc(None, num_devices=NCORES)

    # ---- kernel I/O (per-core; weights pre-packed to SBUF layout on host) ----
    xT_d = nc.dram_tensor("xT", [128, KD, T], BF16, kind="ExternalInput")
    ug_d = nc.dram_tensor("ug_hc", [128, HC, KD * 256], BF16, kind="ExternalInput")
    pre_d = nc.dram_tensor("pre_wT", [128, KD, A], BF16, kind="ExternalInput")
    post_d = nc.dram_tensor("post_wT", [128, A, HC], BF16, kind="ExternalInput")
    pk1_d = nc.dram_tensor("pk1", [128, SC * A + E * A], BF16, kind="ExternalInput")
    ewanc_d = nc.dram_tensor("ewanc", [128, SC * E + 2], F32, kind="ExternalInput")
    wfin_d = nc.dram_tensor("wfin", [128, DC, (HC + 2) * 128], BF16, kind="ExternalInput")
    angb_d = nc.dram_tensor("angb", [2, A], F32, kind="ExternalInput")
    ag_d = nc.dram_tensor("ag_row", [1, A * E], BF16, kind="ExternalInput")
    out_d = nc.dram_tensor("out", [D, T], BF16, kind="ExternalOutput")

    # ---- collective bounce buffers (internal DRAM) ----
    ag1_in = nc.dram_tensor("ag1_in", [T, A], BF16)
    ag1_out = nc.dram_tensor("ag1_out", [S, A], BF16)
    ag2_in = nc.dram_tensor("ag2_in", [A, T], BF16)
    ag2_out = nc.dram_tensor("ag2_out", [GROUP * A, T], BF16)
    RG = [[0, 1, 2, 3], [4, 5, 6, 7]]

    with tile.TileContext(nc) as tc:
        with (
            tc.tile_pool(name="consts", bufs=1) as consts,
            tc.tile_pool(name="wpool", bufs=1) as wpool,
            tc.tile_pool(name="acts", bufs=1) as acts,
            tc.tile_pool(name="work", bufs=4) as work,
            tc.tile_pool(name="work2", bufs=2) as work2,
            tc.tile_pool(name="wtp", bufs=3) as wtp,
            tc.tile_pool(name="workbig", bufs=1) as workbig,
            tc.tile_pool(name="aoln", bufs=1) as aoln,
            tc.tile_pool(name="evac", bufs=4) as evac,
            tc.tile_pool(name="ps_big", bufs=4, space="PSUM") as ps_big,
            tc.tile_pool(name="ps_po", bufs=1, space="PSUM") as ps_po,
            tc.tile_pool(name="ps_out", bufs=2, space="PSUM") as ps_out,
            tc.tile_pool(name="ps_acc", bufs=1, space="PSUM") as ps_acc,
        ):
            # ---------- DMA issue order (sync queue): the pre-matmul chain
            # first, then up/gate weights interleaved with packed consts.
            # Each dma_start costs ~0.6us of queue issue time, so consolidate.
            # scalar queue carries the small early loads so they overlap the
            # sync queue's xT/ug stream on the SDMA rings
            pre_w = wpool.tile([128, KD, A], BF16)
            nc.scalar.dma_start(out=pre_w, in_=pre_d[:])
            xT = wpool.tile([128, KD, T], BF16)
            nc.sync.dma_start(out=xT[:, 0:2, :], in_=xT_d[:, 0:2, :])
            nc.sync.dma_start(out=xT[:, 2:4, :], in_=xT_d[:, 2:4, :])
            nc.scalar.dma_start(out=xT[:, 4:6, :], in_=xT_d[:, 4:6, :])
            nc.scalar.dma_start(out=xT[:, 6:8, :], in_=xT_d[:, 6:8, :])
            ug_w = wpool.tile([128, HC, KD, 256], BF16, tag="bigw")
            nc.sync.dma_start(out=ug_w[:, 0:2], in_=ug_d[:, 0:2, :])
            gB = consts.tile([128, A], F32)   # an_g broadcast across partitions
            bB = consts.tile([128, A], F32)   # an_b broadcast
            nc.scalar.dma_start(
                out=gB,
                in_=bass.AP(tensor=angb_d, offset=0, ap=[[0, 128], [1, A]]),
            )
            nc.scalar.dma_start(
                out=bB,
                in_=bass.AP(tensor=angb_d, offset=A, ap=[[0, 128], [1, A]]),
            )
            angb_sb = consts.tile([2, A], F32)   # raw an_g / an_b rows
            nc.scalar.dma_start(out=angb_sb, in_=angb_d[:])
            ewanc = consts.tile([128, SC * E + 2], F32)    # ew | ancol
            nc.scalar.dma_start(out=ewanc, in_=ewanc_d[:])
            agB = consts.tile([128, E, A], BF16)  # adapter_g (e-major) bcast
            nc.scalar.dma_start(
                out=agB,
                in_=bass.AP(tensor=ag_d, offset=0, ap=[[0, 128], [A, E], [1, A]]),
            )
            pk1 = wpool.tile([128, SC * A + E * A], BF16)  # bias_mix | adapter_wT
            nc.sync.dma_start(out=pk1, in_=pk1_d[:])
            post_w = wpool.tile([128, A, HC], BF16)
            nc.sync.dma_start(out=post_w, in_=post_d[:])
            nc.sync.dma_start(out=ug_w[:, 2:4], in_=ug_d[:, 2:4, :])
            for i in range(1, 4):
                nc.sync.dma_start(
                    out=ug_w[:, 4 * i:4 * i + 4], in_=ug_d[:, 4 * i:4 * i + 4, :]
                )
            wfin = wpool.tile([128, DC, HC + 2, 128], BF16, tag="bigw")

            # ---------- memset constants (vector queue) ----------
            eps_t = consts.tile([128, 1], F32)
            nc.vector.memset(eps_t, EPS)
            ones_col = consts.tile([128, 1], BF16)
            nc.vector.memset(ones_col, 1.0)
            ones_row = consts.tile([1, 128], BF16)
            nc.vector.memset(ones_row, 1.0)
            ident = consts.tile([128, 128], BF16)
            masks.make_identity(nc, ident[:])

            # persistent activations
            AI_tok = acts.tile([128, SC, A], BF16)    # adapt_in, token-major
            AIT = acts.tile([128, T], BF16)           # adapt_in, feature-major
            hid = acts.tile([128, HC, T], BF16)       # hidden, feature-major
            AOTfull = acts.tile([128, GROUP, T], BF16)   # gathered AO feat-major
            AOT = acts.tile([128, T], BF16)           # local AO, feature-major
            adaptT = acts.tile([128, T], BF16)        # adapt, feature-major
            mixedT = acts.tile([128, T], BF16)        # mixed, feature-major
            mix_tok = acts.tile([128, SC, A], BF16)   # mixed, token-major
            facc = acts.tile([128, DC, T], BF16)      # down+mixed partial acc

            def layernorm_to(ps, dst):
                """LN over free dim (A=128) of psum tile [128, A]; write dst bf16."""
                st = work.tile([128, 6], F32, tag="lnst")
                nc.vector.bn_stats(out=st, in_=ps)
                mv = work.tile([128, 2], F32, tag="lnmv")
                nc.vector.bn_aggr(out=mv, in_=st)
                sd = work.tile([128, 1], F32, tag="lnsd")
                nc.scalar.activation(
                    out=sd, in_=mv[:, 1:2], func=mybir.ActivationFunctionType.Sqrt,
                    bias=eps_t, scale=1.0,
                )
                r = work.tile([128, 1], F32, tag="lnr")
                nc.vector.reciprocal(out=r, in_=sd)
                z = work.tile([128, A], F32, tag="lnz")
                nc.vector.tensor_scalar(
                    out=z, in0=ps, scalar1=mv[:, 0:1], scalar2=r,
                    op0=mybir.AluOpType.subtract, op1=mybir.AluOpType.mult,
                )
                zg = work.tile([128, A], F32, tag="lnzg")
                nc.vector.tensor_tensor(out=zg, in0=z, in1=gB, op=mybir.AluOpType.mult)
                nc.vector.tensor_tensor(out=dst, in0=zg, in1=bB, op=mybir.AluOpType.add)

            # ---------- adapt_in = LN(x @ pre_w.T): k-outer so the 4 psum
            # groups stream against the xT DMA chunks as they arrive.
            pre_ps = [
                ps_big.tile([128, A], F32, tag="mm", name=f"pre_ps{sc}")
                for sc in range(SC)
            ]
            for k in range(KD):
                for sc in range(SC):
                    nc.tensor.matmul(
                        pre_ps[sc], xT[:, k, sc * 128:(sc + 1) * 128], pre_w[:, k, :],
                        start=(k == 0), stop=(k == KD - 1),
                    )
            for sc in range(SC):
                layernorm_to(pre_ps[sc], AI_tok[:, sc, :])

            # AIT transposes via the PE (ring-independent: a pending
            # collective freezes DMA rings, so transposes must not ride them).
            for sc in range(SC):
                tp = ps_out.tile([128, 128], BF16, tag="fout")
                nc.tensor.transpose(tp, AI_tok[:, sc, :], ident[:])
                if sc % 2 == 0:
                    nc.scalar.copy(out=AIT[:, sc * 128:(sc + 1) * 128], in_=tp)
                else:
                    nc.vector.tensor_copy(
                        out=AIT[:, sc * 128:(sc + 1) * 128], in_=tp
                    )
            # wfin, then AG1 staging LAST on the sync ring: ring FIFO puts the
            # staging transfer after every weight transfer, so the pending
            # collective can never starve the weight stream (v3 lost 17us to
            # exactly that; v4 lost 24us to the transposes queued behind it).
            nc.sync.dma_start(out=wfin[:, 0:4], in_=wfin_d[:, 0:4, :])
            nc.sync.dma_start(out=wfin[:, 4:8], in_=wfin_d[:, 4:8, :])
            nc.sync.dma_start(
                out=ag1_in[:].rearrange("(sc p) a -> p sc a", p=128), in_=AI_tok
            )
            nc.gpsimd.collective_compute(
                "AllGather", mybir.AluOpType.bypass, replica_groups=RG,
                ins=[ag1_in[:]], outs=[ag1_out[:]],
            )
            AIfull = acts.tile([128, TC_FULL, A], BF16)   # gathered AI token-major
            nc.gpsimd.dma_start(
                out=AIfull, in_=ag1_out[:].rearrange("(k p) a -> p k a", p=128)
            )

            # ---------- expert path, part 1 (injected into the up/gate loop) ---
            hsb = acts.tile([128, SC, E * A], BF16)   # h, token-major, e-outer
            mv8s = work.tile([128, SC, E, 2], F32, tag="mv8s", bufs=1)

            def expert_mm(sc):
                hp0 = ps_out.tile([128, 512], F32, tag="fout")
                hp1 = ps_out.tile([128, 512], F32, tag="fout")
                sl = AIT[:, sc * 128:(sc + 1) * 128]
                nc.tensor.matmul(hp0, sl, pk1[:, SC * A:SC * A + 512],
                                 start=True, stop=True)
                nc.tensor.matmul(hp1, sl, pk1[:, SC * A + 512:SC * A + 1024],
                                 start=True, stop=True)
                st8 = work.tile([128, E, 6], F32, tag="st8")
                hps = [hp0, hp0, hp0, hp0, hp1, hp1, hp1, hp1]
                for e in range(E):
                    nc.vector.bn_stats(
                        out=st8[:, e, :], in_=hps[e][:, (e % 4) * A:(e % 4 + 1) * A]
                    )
                for e in range(E):
                    nc.vector.bn_aggr(out=mv8s[:, sc, e, :], in_=st8[:, e, :])
                nc.scalar.copy(out=hsb[:, sc, 0:512], in_=hp0)
                nc.scalar.copy(out=hsb[:, sc, 512:1024], in_=hp1)

            def expert_ln(sc, r8all):
                rw8 = work.tile([128, E], F32, tag="rw8")
                nc.vector.tensor_tensor(
                    out=rw8, in0=r8all[:, sc, :],
                    in1=ewanc[:, sc * E:(sc + 1) * E],
                    op=mybir.AluOpType.mult,
                )
                nmrw = work.tile([128, E], F32, tag="nmrw")
                nc.vector.tensor_tensor(
                    out=nmrw, in0=mv8s[:, sc, :, 0], in1=rw8,
                    op=mybir.AluOpType.mult,
                )
                nc.vector.tensor_scalar(
                    out=nmrw, in0=nmrw, scalar1=-1.0, scalar2=None,
                    op0=mybir.AluOpType.mult,
                )
                zt = workbig.tile([128, E, A], BF16, tag="zt")
                for e in range(E):
                    nc.vector.tensor_scalar(
                        out=zt[:, e, :], in0=hsb[:, sc, e * A:(e + 1) * A],
                        scalar1=rw8[:, e:e + 1], scalar2=nmrw[:, e:e + 1],
                        op0=mybir.AluOpType.mult, op1=mybir.AluOpType.add,
                    )
                zg = workbig.tile([128, E, A], BF16, tag="ztg")
                nc.vector.tensor_tensor(
                    out=zg, in0=zt, in1=agB, op=mybir.AluOpType.mult
                )
                t1 = workbig.tile([128, 4, A], BF16, tag="sum1")
                nc.vector.tensor_tensor(
                    out=t1, in0=zg[:, 0:4, :], in1=zg[:, 4:8, :],
                    op=mybir.AluOpType.add,
                )
                t2 = work.tile([128, 2, A], BF16, tag="sum2")
                nc.vector.tensor_tensor(
                    out=t2, in0=t1[:, 0:2, :], in1=t1[:, 2:4, :],
                    op=mybir.AluOpType.add,
                )
                mx = work.tile([128, A], BF16, tag="mx")
                nc.vector.tensor_tensor(
                    out=mx, in0=t2[:, 0, :], in1=t2[:, 1, :], op=mybir.AluOpType.add
                )
                nc.vector.tensor_tensor(
                    out=mix_tok[:, sc, :], in0=mx,
                    in1=pk1[:, sc * A:(sc + 1) * A],
                    op=mybir.AluOpType.add,
                )

            # ---------- hidden = silu(x@gate.T) * (x@up.T), feature-major -------
            po_ps = ps_po.tile([128, T], F32, tag="po")

            def post_step(k):
                nc.tensor.matmul(
                    po_ps, post_w[:, :, k], hid[:, k, :],
                    start=(k == 0), stop=(k == HC - 1),
                )

            for hc in range(HC):
                up_ps = ps_big.tile([128, T], F32, tag="mm")
                gt_ps = ps_big.tile([128, T], F32, tag="mm")
                for k in range(KD):
                    nc.tensor.matmul(
                        up_ps, ug_w[:, hc, k, 0:128], xT[:, k, :],
                        start=(k == 0), stop=(k == KD - 1),
                    )
                for k in range(KD):
                    nc.tensor.matmul(
                        gt_ps, ug_w[:, hc, k, 128:256], xT[:, k, :],
                        start=(k == 0), stop=(k == KD - 1),
                    )
                sg = work2.tile([128, T], BF16, tag="sg")
                nc.scalar.activation(
                    out=sg, in_=gt_ps, func=mybir.ActivationFunctionType.Silu
                )
                nc.vector.tensor_tensor(
                    out=hid[:, hc, :], in0=sg, in1=up_ps, op=mybir.AluOpType.mult
                )
                if hc > 0:
                    post_step(hc - 1)
                if 3 <= hc <= 6:
                    expert_mm(hc - 3)
            post_step(HC - 1)

            # ---------- expert path, part 2 (runs mid-up/gate off mv8s) ------
            sd8all = work.tile([128, SC, E], F32, tag="sd8all", bufs=1)
            nc.scalar.activation(
                out=sd8all, in_=mv8s[:, :, :, 1],
                func=mybir.ActivationFunctionType.Sqrt, bias=eps_t, scale=1.0,
            )
            r8all = work.tile([128, SC, E], F32, tag="r8all", bufs=1)
            nc.vector.reciprocal(out=r8all, in_=sd8all)
            for sc in range(SC):
                expert_ln(sc, r8all)
            # mixed -> feature-major via PE transposes (ring-independent)
            for sc in range(SC):
                tp = ps_out.tile([128, 128], BF16, tag="fout")
                nc.tensor.transpose(tp, mix_tok[:, sc, :], ident[:])
                if sc % 2 == 0:
                    nc.scalar.copy(out=mixedT[:, sc * 128:(sc + 1) * 128], in_=tp)
                else:
                    nc.vector.tensor_copy(
                        out=mixedT[:, sc * 128:(sc + 1) * 128], in_=tp
                    )

            # ---------- adapt_out LN, feature-major ----------
            # stats via PE ones-matmuls; rstd via reciprocal_approx_fast on the
            # [1,T] row (1-lane DVE but only ~0.7us); broadcast back over
            # partitions with two K=1 ones-row matmuls into PSUM.
            AOf = acts.tile([128, T], BF16)
            nc.vector.tensor_copy(out=AOf, in_=po_ps)
            # dummy Sqrt pinned to the last hid chunk: forces the act-table
            # switch while the AOf evacuation runs on vector, so the real sqrt
            # below hits a warm table
            tdum = aoln.tile([1, 1], F32)
            nc.scalar.activation(
                out=tdum, in_=hid[0:1, 15, 0:1],
                func=mybir.ActivationFunctionType.Sqrt,
                bias=eps_t[0:1], scale=1.0,
            )
            sqf = aoln.tile([128, T], BF16)
            nc.vector.tensor_tensor(out=sqf, in0=AOf, in1=AOf, op=mybir.AluOpType.mult)
            s0 = ps_po.tile([1, T], F32, tag="po")
            nc.tensor.matmul(s0, ones_col, AOf, start=True, stop=True)
            s1 = ps_acc.tile([1, T], F32, tag="adps")
            nc.tensor.matmul(s1, ones_col, sqf, start=True, stop=True)
            # stats split across scalar (mean, mean^2) and vector (E[x^2],
            # var, rstd, mean*rstd); an_g is folded into the K=1 broadcast
            # matmuls (lhsT = raw an_g row), so the token-space transform is
            # two fused vector ops.
            mean_b = aoln.tile([1, T], F32)
            nc.scalar.activation(
                out=mean_b, in_=s0, func=mybir.ActivationFunctionType.Copy,
                scale=1.0 / A,
            )
            e2 = aoln.tile([1, T], F32)
            nc.vector.tensor_scalar(
                out=e2, in0=s1, scalar1=1.0 / A, scalar2=None,
                op0=mybir.AluOpType.mult,
            )
            m2 = aoln.tile([1, T], F32)
            nc.scalar.square(out=m2, in_=mean_b)
            varf = aoln.tile([1, T], F32)
            nc.vector.tensor_tensor(out=varf, in0=e2, in1=m2,
                                    op=mybir.AluOpType.subtract)
            sdf = aoln.tile([1, T], F32)
            nc.scalar.activation(
                out=sdf, in_=varf, func=mybir.ActivationFunctionType.Sqrt,
                bias=eps_t[0:1], scale=1.0,
            )
            r32 = aoln.tile([1, T], F32)
            nc.vector.reciprocal_approx_fast(out=r32, in_=sdf)
            mr32 = aoln.tile([1, T], F32)   # mean * rstd (subtracted later)
            nc.vector.tensor_tensor(out=mr32, in0=mean_b, in1=r32,
                                    op=mybir.AluOpType.mult)
            # half of fd(0) fills the PE while the rstd row chain runs
            fd0_ps = ps_out.tile([128, T], F32, tag="fout", name="fd0_ps")
            for k in range(8):
                nc.tensor.matmul(
                    fd0_ps, wfin[:, 0, k, :], hid[:, k, :],
                    start=(k == 0), stop=False,
                )
            grB_ps = ps_big.tile([128, T], F32, tag="mm", name="grB_ps")
            nc.tensor.matmul(grB_ps, angb_sb[0:1, :], r32, start=True, stop=True)
            gmrB_ps = ps_big.tile([128, T], F32, tag="mm", name="gmrB_ps")
            nc.tensor.matmul(gmrB_ps, angb_sb[0:1, :], mr32, start=True, stop=True)
            z1 = aoln.tile([128, T], BF16)
            nc.vector.tensor_tensor(out=z1, in0=AOf, in1=grB_ps,
                                    op=mybir.AluOpType.mult)
            nc.vector.scalar_tensor_tensor(
                out=AOT, in0=z1, scalar=ewanc[:, SC * E + 1:SC * E + 2],
                in1=gmrB_ps, op0=mybir.AluOpType.add, op1=mybir.AluOpType.subtract,
            )
            # stage + trigger AG2
            nc.scalar.dma_start(out=ag2_in[:], in_=AOT)
            nc.gpsimd.collective_compute(
                "AllGather", mybir.AluOpType.bypass, replica_groups=RG,
                ins=[ag2_in[:]], outs=[ag2_out[:]],
            )
            # dummy Silu pinned to AOT: warm the Silu table for the bmm1 silus
            tdum2 = aoln.tile([1, 1], BF16)
            nc.scalar.activation(
                out=tdum2, in_=AOT[0:1, 0:1], func=mybir.ActivationFunctionType.Silu,
            )
            for c in range(GROUP):
                nc.gpsimd.dma_start(
                    out=AOTfull[:, c, :], in_=ag2_out[c * A:(c + 1) * A, :]
                )
            AOTf = AOTfull.rearrange("a c t -> a (c t)")

            # ---------- final_down (down contribution only; mixed+adapt land
            # in final_close so fd never waits on the expert path) ----------
            fd_open = {}

            def final_down(dc):
                if dc == 0:
                    op = fd0_ps
                    ks = range(8, HC)
                else:
                    op = ps_out.tile([128, T], F32, tag="fout")
                    ks = range(HC)
                for k in ks:
                    nc.tensor.matmul(
                        op, wfin[:, dc, k, :], hid[:, k, :],
                        start=(k == 0), stop=(k == HC - 1),
                    )
                if dc >= DC - 2:
                    # last two banks stay resident; final_close accumulates
                    # adapt+mixed on top and a scalar copy evacuates
                    fd_open[dc] = op
                elif dc % 2 == 0:
                    nc.scalar.copy(out=facc[:, dc, :], in_=op)
                else:
                    nc.vector.tensor_copy(out=facc[:, dc, :], in_=op)

            for dc in range(DC):
                final_down(dc)

            # ---------- w = silu(clip(AI_loc @ AO_full.T)); adapt = w.T chain ---
            ad_ps = ps_acc.tile([128, T], F32, tag="adps")
            wts_buf = {}

            def bmm1_step(j):
                w_ps = ps_big.tile([128, T], F32, tag="mm")
                nc.tensor.matmul(
                    w_ps, AOTf[:, j * 128:(j + 1) * 128], AIT, start=True, stop=True
                )
                wc = work2.tile([128, T], BF16, tag="wc")
                nc.vector.tensor_scalar(
                    out=wc, in0=w_ps, scalar1=-5.0, scalar2=5.0,
                    op0=mybir.AluOpType.max, op1=mybir.AluOpType.min,
                )
                wt = wtp.tile([128, T], BF16, tag="wts")
                nc.scalar.activation(
                    out=wt, in_=wc, func=mybir.ActivationFunctionType.Silu
                )
                wts_buf[j] = wt

            # depth-2 software pipeline: bmm2(j) trails bmm1(j+2) so the PE
            # never waits on the clip/silu stages
            for j in range(TC_FULL):
                bmm1_step(j)
                if j >= 2:
                    nc.tensor.matmul(
                        ad_ps, AIfull[:, j - 2, :], wts_buf.pop(j - 2),
                        start=(j - 2 == 0), stop=False,
                    )
            for j in (TC_FULL - 2, TC_FULL - 1):
                nc.tensor.matmul(
                    ad_ps, AIfull[:, j, :], wts_buf.pop(j),
                    start=False, stop=(j == TC_FULL - 1),
                )
            nc.scalar.copy(out=adaptT, in_=ad_ps)

            # ---------- finish output ----------
            def final_close(dc):
                if dc in fd_open:
                    op = fd_open[dc]
                    nc.tensor.matmul(
                        op, wfin[:, dc, HC, :], adaptT,
                        start=False, stop=False, skip_group_check=True,
                    )
                    nc.tensor.matmul(
                        op, wfin[:, dc, HC + 1, :], mixedT,
                        start=False, stop=True, skip_group_check=True,
                    )
                    ob = evac.tile([128, T], BF16, tag="ob")
                    nc.scalar.copy(out=ob, in_=op)
                else:
                    op = ps_big.tile([128, T], F32, tag="mm")
                    nc.tensor.matmul(
                        op, wfin[:, dc, HC, :], adaptT, start=True, stop=False,
                    )
                    nc.tensor.matmul(
                        op, wfin[:, dc, HC + 1, :], mixedT, start=False, stop=True,
                    )
                    ob = evac.tile([128, T], BF16, tag="ob")
                    nc.vector.tensor_tensor(
                        out=ob, in0=facc[:, dc, :], in1=op, op=mybir.AluOpType.add
                    )
                nc.sync.dma_start(out=out_d[dc * 128:(dc + 1) * 128, :], in_=ob)

            for dc in range(DC):
                final_close(dc)

    nc.compile()
    return nc


def kernel(
    x, expert_weights, up_w, gate_w, down_w, pre_w, post_w, an_g, an_b,
    adapt_proj_w, adapter_w, adapter_g, adapter_b, expert_proj_w, output_proj_w,
):
    x = np.asarray(x, np.float32)
    expert_weights = np.asarray(expert_weights, np.float32)
    bf = ml_dtypes.bfloat16

    if "nc" not in _CACHE:
        _CACHE["nc"] = _build()
    nc = _CACHE["nc"]

    def pack(w, kc):
        # [kc*128, F] -> [128, kc, F] (partition-major SBUF layout)
        f = w.shape[1]
        return np.ascontiguousarray(
            w.reshape(kc, 128, f).transpose(1, 0, 2)
        ).astype(bf)

    ug_wT = np.concatenate(
        [np.asarray(up_w, np.float32), np.asarray(gate_w, np.float32)], axis=0
    ).T                                                        # [D, 2H]
    # hc-major pack: [128, HC, KD*256]; per hc: KD chunks of (up 128 | gate 128)
    up_part = ug_wT[:, :H].reshape(KD, 128, HC, 128)
    gt_part = ug_wT[:, H:].reshape(KD, 128, HC, 128)
    ug_hc = np.ascontiguousarray(
        np.stack([up_part, gt_part], axis=3)        # [k, p, hc, 2, 128]
        .transpose(1, 2, 0, 3, 4)                   # [p, hc, k, 2, 128]
        .reshape(128, HC, KD * 256)
    ).astype(bf)
    pre_wT = np.asarray(pre_w, np.float32).T                   # [D, A]
    post_pack = np.ascontiguousarray(
        np.asarray(post_w, np.float32).T.reshape(HC, 128, A).transpose(1, 2, 0)
    ).astype(bf)                                               # [128, A, HC]
    adapter_wT = (
        np.asarray(adapter_w, np.float32).transpose(2, 0, 1).reshape(A, E * A)
    ).astype(bf)                                               # [A, E*A] (e-major)
    down_w = np.asarray(down_w, np.float32)
    w_da = 0.1 * (down_w @ np.asarray(adapt_proj_w, np.float32))       # [D, A]
    w_mo = np.asarray(output_proj_w, np.float32) @ np.asarray(
        expert_proj_w, np.float32
    )                                                                   # [D, A]
    wfin = np.concatenate([down_w.T, w_da.T, w_mo.T], axis=0)  # [2304, D]
    angb = np.stack(
        [np.asarray(an_g, np.float32), np.asarray(an_b, np.float32)], axis=0
    )                                                                   # [2, A]
    ancol = np.ascontiguousarray(angb.T)                                # [A, 2]
    ag_row = np.asarray(adapter_g, np.float32).reshape(1, A * E).astype(bf)  # e-major
    bias_mix = (expert_weights @ np.asarray(adapter_b, np.float32)).astype(bf)

    xf = x.reshape(N, D)
    shared = {
        "ug_hc": ug_hc, "pre_wT": pack(pre_wT, KD),
        "post_wT": post_pack,
        "wfin": np.ascontiguousarray(
            wfin.reshape(HC + 2, 128, DC, 128).transpose(1, 2, 0, 3)
            .reshape(128, DC, (HC + 2) * 128)
        ).astype(bf), "angb": angb,
        "ag_row": ag_row,
    }
    in_maps = []
    for c in range(NCORES):
        sl = slice(c * T, (c + 1) * T)
        ewc = np.ascontiguousarray(expert_weights[sl]).reshape(SC, 128, E)
        ew_c = np.ascontiguousarray(ewc.transpose(1, 0, 2)).reshape(128, SC * E)
        bmc = np.ascontiguousarray(bias_mix[sl]).reshape(SC, 128, A)
        bm_c = np.ascontiguousarray(bmc.transpose(1, 0, 2)).reshape(128, SC * A)
        pk1_c = np.concatenate(
            [bm_c, np.broadcast_to(adapter_wT, (128, E * A)).astype(bf)], axis=1
        )
        ewanc_c = np.concatenate(
            [ew_c.astype(np.float32),
             np.broadcast_to(ancol.reshape(A, 2), (128, 2)).astype(np.float32)],
            axis=1,
        )
        in_maps.append(
            dict(
                shared,
                xT=pack(np.ascontiguousarray(xf[sl].T), KD),
                pk1=np.ascontiguousarray(pk1_c),
                ewanc=np.ascontiguousarray(ewanc_c),
            )
        )

    try:
        res = run_bass_kernel_spmd(nc, in_maps, list(range(NCORES))).results
    except Exception:
        # axon workers occasionally hang up; one retry on a fresh dispatch
        import time

        time.sleep(10)
        res = run_bass_kernel_spmd(nc, in_maps, list(range(NCORES))).results
    out = np.empty((N, D), np.float32)
    for c in range(NCORES):
        out[c * T:(c + 1) * T] = np.asarray(res[c]["out"], np.float32).T
    return out.reshape(B, S, D)


# revision 16
# speedup vs baseline: 1.0558x; 1.0231x over previous
"""Trainium2 Bass kernel for nn_ExpertGroup (moe_routing).

Contract: kernel(**inputs) takes FULL unsharded numpy inputs and returns the
FULL [2, 2048, 1024] fp32 output. Internally shards B*S=4096 tokens across
8 NeuronCores (512 tokens/core; cores 0-3 own batch 0, cores 4-7 batch 1),
replicates the small weights, and exchanges the per-batch adapter tensors
(adapt_in / adapt_out, [S,128] each) with two intra-group AllGathers.

All matmuls run in bf16 with fp32 PSUM accumulation. Activations are laid out
feature-major ([feature, token]) so every matmul contracts over partitions.

v3 schedule (vs v2 at 182us). Trace findings v2: 11us dead lead-in (DMA issue
cost ~0.6us each x 41 issues), 4us PE gap at 31us (AIT xbar transposes jam
the scalar queue ahead of the up/gate silu evacuations), AO-LN tail takes
29.5us (DRAM-bounce reciprocal dance) so AG2 triggers at 131.5us and the PE
idles 9.4us + re-warms cold. Changes:
  - DMA consolidation: 16 issues instead of 41 (xT in 4, consts packed into
    pk1/ewanc, ug in 6, wfin in 2, own SBUF slot - no WAR streaming games)
  - pre-GEMM k-outer so the 4 psum groups stream against the xT DMA
  - AIT transposes moved to the gpsimd queue (idle until the AG1 trigger)
  - AO-LN tail: stats via ones-matmul (as v2) but rstd via
    reciprocal_approx_fast on the [1,T] row (no DRAM bounce, no
    re-partition), broadcast back via two K=1 ones-row matmuls into PSUM
  - act-table hoists: dummy Sqrt/Silu activations pinned to AOf/AOT
    readiness so the real AO sqrt and the bmm1 silus hit a warm table
  - mixedT contribution folded into the final_down accumulation (ready at
    ~65us), so final_close is one matmul per dc
  - facc in bf16, evacuations alternate scalar/vector
"""

import sys

sys.path.insert(0, "/opt/trn_rl_repo")

import ml_dtypes
import numpy as np

import concourse.bass as bass
import concourse.mybir as mybir
import concourse.tile as tile
from concourse import masks
from concourse import bacc
from concourse.bass_utils import run_bass_kernel_spmd

BF16 = mybir.dt.bfloat16
F32 = mybir.dt.float32

B, S, D, E = 2, 2048, 1024, 8
H = 2 * D          # 2048
A = H // 16        # 128
N = B * S          # 4096
NCORES = 8
T = N // NCORES    # 512 tokens per core
GROUP = 4          # cores per batch
SC = T // 128      # 4 s-chunks per core
DC = D // 128      # 8 d-chunks (output features)
HC = H // 128      # 16 h-chunks
KD = D // 128      # 8 k-chunks over D
TC_FULL = S // 128  # 16 token-chunks per batch
EPS = 1e-5

_CACHE = {}


def _build():
    nc = bacc.Bacc(None, num_devices=NCORES)

    # ---- kernel I/O (per-core; weights pre-packed to SBUF layout on host) ----
    xT_d = nc.dram_tensor("xT", [128, KD, T], BF16, kind="ExternalInput")
    ug_d = nc.dram_tensor("ug_hc", [128, HC, KD * 256], BF16, kind="ExternalInput")
    pre_d = nc.dram_tensor("pre_wT", [128, KD, A], BF16, kind="ExternalInput")
    post_d = nc.dram_tensor("post_wT", [128, A, HC], BF16, kind="ExternalInput")
    pk1_d = nc.dram_tensor("pk1", [128, SC * A + E * A], BF16, kind="ExternalInput")
    ewanc_d = nc.dram_tensor("ewanc", [128, SC * E + 2], F32, kind="ExternalInput")
    wfin_d = nc.dram_tensor("wfin", [128, DC, (HC + 2) * 128], BF16, kind="ExternalInput")
    angb_d = nc.dram_tensor("angb", [2, A], F32, kind="ExternalInput")
    ag_d = nc.dram_tensor("ag_row", [1, A * E], BF16, kind="ExternalInput")
    out_d = nc.dram_tensor("out", [D, T], BF16, kind="ExternalOutput")

    # ---- collective bounce buffers (internal DRAM) ----
    ag1_in = nc.dram_tensor("ag1_in", [T, A], BF16)
    ag1_out = nc.dram_tensor("ag1_out", [S, A], BF16)
    ag2_in = nc.dram_tensor("ag2_in", [A, T], BF16)
    ag2_out = nc.dram_tensor("ag2_out", [GROUP * A, T], BF16)
    RG = [[0, 1, 2, 3], [4, 5, 6, 7]]

    with tile.TileContext(nc) as tc:
        with (
            tc.tile_pool(name="consts", bufs=1) as consts,
            tc.tile_pool(name="wpool", bufs=1) as wpool,
            tc.tile_pool(name="acts", bufs=1) as acts,
            tc.tile_pool(name="work", bufs=4) as work,
            tc.tile_pool(name="work2", bufs=2) as work2,
            tc.tile_pool(name="wtp", bufs=3) as wtp,
            tc.tile_pool(name="workbig", bufs=1) as workbig,
            tc.tile_pool(name="aoln", bufs=1) as aoln,
            tc.tile_pool(name="evac", bufs=4) as evac,
            tc.tile_pool(name="ps_big", bufs=4, space="PSUM") as ps_big,
            tc.tile_pool(name="ps_po", bufs=1, space="PSUM") as ps_po,
            tc.tile_pool(name="ps_out", bufs=2, space="PSUM") as ps_out,
            tc.tile_pool(name="ps_acc", bufs=1, space="PSUM") as ps_acc,
        ):
            # ---------- DMA issue order (sync queue): the pre-matmul chain
            # first, then up/gate weights interleaved with packed consts.
            # Each dma_start costs ~0.6us of queue issue time, so consolidate.
            # scalar queue carries the small early loads so they overlap the
            # sync queue's xT/ug stream on the SDMA rings
            pre_w = wpool.tile([128, KD, A], BF16)
            nc.scalar.dma_start(out=pre_w, in_=pre_d[:])
            xT = wpool.tile([128, KD, T], BF16)
            nc.sync.dma_start(out=xT[:, 0:2, :], in_=xT_d[:, 0:2, :])
            nc.sync.dma_start(out=xT[:, 2:4, :], in_=xT_d[:, 2:4, :])
            nc.scalar.dma_start(out=xT[:, 4:6, :], in_=xT_d[:, 4:6, :])
            nc.scalar.dma_start(out=xT[:, 6:8, :], in_=xT_d[:, 6:8, :])
            ug_w = wpool.tile([128, HC, KD, 256], BF16, tag="bigw")
            nc.sync.dma_start(out=ug_w[:, 0:2], in_=ug_d[:, 0:2, :])
            gB = consts.tile([128, A], F32)   # an_g broadcast across partitions
            bB = consts.tile([128, A], F32)   # an_b broadcast
            nc.scalar.dma_start(
                out=gB,
                in_=bass.AP(tensor=angb_d, offset=0, ap=[[0, 128], [1, A]]),
            )
            nc.scalar.dma_start(
                out=bB,
                in_=bass.AP(tensor=angb_d, offset=A, ap=[[0, 128], [1, A]]),
            )
            angb_sb = consts.tile([2, A], F32)   # raw an_g / an_b rows
            nc.scalar.dma_start(out=angb_sb, in_=angb_d[:])
            ewanc = consts.tile([128, SC * E + 2], F32)    # ew | ancol
            nc.scalar.dma_start(out=ewanc, in_=ewanc_d[:])
            agB = consts.tile([128, E, A], BF16)  # adapter_g (e-major) bcast
            nc.scalar.dma_start(
                out=agB,
                in_=bass.AP(tensor=ag_d, offset=0, ap=[[0, 128], [A, E], [1, A]]),
            )
            pk1 = wpool.tile([128, SC * A + E * A], BF16)  # bias_mix | adapter_wT
            nc.sync.dma_start(out=pk1, in_=pk1_d[:])
            post_w = wpool.tile([128, A, HC], BF16)
            nc.sync.dma_start(out=post_w, in_=post_d[:])
            nc.sync.dma_start(out=ug_w[:, 2:4], in_=ug_d[:, 2:4, :])
            for i in range(1, 4):
                nc.sync.dma_start(
                    out=ug_w[:, 4 * i:4 * i + 4], in_=ug_d[:, 4 * i:4 * i + 4, :]
                )
            wfin = wpool.tile([128, DC, HC + 2, 128], BF16, tag="bigw")

            # ---------- memset constants (vector queue) ----------
            eps_t = consts.tile([128, 1], F32)
            nc.vector.memset(eps_t, EPS)
            ones_col = consts.tile([128, 1], BF16)
            nc.vector.memset(ones_col, 1.0)
            ones_row = consts.tile([1, 128], BF16)
            nc.vector.memset(ones_row, 1.0)
            ident = consts.tile([128, 128], BF16)
            masks.make_identity(nc, ident[:])

            # persistent activations
            AI_tok = acts.tile([128, SC, A], BF16)    # adapt_in, token-major
            AIT = acts.tile([128, T], BF16)           # adapt_in, feature-major
            hid = acts.tile([128, HC, T], BF16)       # hidden, feature-major
            AOTfull = acts.tile([128, GROUP, T], BF16)   # gathered AO feat-major
            AOT = acts.tile([128, T], BF16)           # local AO, feature-major
            adaptT = acts.tile([128, T], BF16)        # adapt, feature-major
            mixedT = acts.tile([128, T], BF16)        # mixed, feature-major
            mix_tok = acts.tile([128, SC, A], BF16)   # mixed, token-major
            facc = acts.tile([128, DC, T], BF16)      # down+mixed partial acc

            def layernorm_to(ps, dst):
                """LN over free dim (A=128) of psum tile [128, A]; write dst bf16."""
                st = work.tile([128, 6], F32, tag="lnst")
                nc.vector.bn_stats(out=st, in_=ps)
                mv = work.tile([128, 2], F32, tag="lnmv")
                nc.vector.bn_aggr(out=mv, in_=st)
                sd = work.tile([128, 1], F32, tag="lnsd")
                nc.scalar.activation(
                    out=sd, in_=mv[:, 1:2], func=mybir.ActivationFunctionType.Sqrt,
                    bias=eps_t, scale=1.0,
                )
                r = work.tile([128, 1], F32, tag="lnr")
                nc.vector.reciprocal(out=r, in_=sd)
                z = work.tile([128, A], F32, tag="lnz")
                nc.vector.tensor_scalar(
                    out=z, in0=ps, scalar1=mv[:, 0:1], scalar2=r,
                    op0=mybir.AluOpType.subtract, op1=mybir.AluOpType.mult,
                )
                zg = work.tile([128, A], F32, tag="lnzg")
                nc.vector.tensor_tensor(out=zg, in0=z, in1=gB, op=mybir.AluOpType.mult)
                nc.vector.tensor_tensor(out=dst, in0=zg, in1=bB, op=mybir.AluOpType.add)

            # ---------- adapt_in = LN(x @ pre_w.T): k-outer so the 4 psum
            # groups stream against the xT DMA chunks as they arrive.
            pre_ps = [
                ps_big.tile([128, A], F32, tag="mm", name=f"pre_ps{sc}")
                for sc in range(SC)
            ]
            for k in range(KD):
                for sc in range(SC):
                    nc.tensor.matmul(
                        pre_ps[sc], xT[:, k, sc * 128:(sc + 1) * 128], pre_w[:, k, :],
                        start=(k == 0), stop=(k == KD - 1),
                    )
            for sc in range(SC):
                layernorm_to(pre_ps[sc], AI_tok[:, sc, :])

            # AIT transposes via the PE (ring-independent: a pending
            # collective freezes DMA rings, so transposes must not ride them).
            for sc in range(SC):
                tp = ps_out.tile([128, 128], BF16, tag="fout")
                nc.tensor.transpose(tp, AI_tok[:, sc, :], ident[:])
                if sc % 2 == 0:
                    nc.scalar.copy(out=AIT[:, sc * 128:(sc + 1) * 128], in_=tp)
                else:
                    nc.vector.tensor_copy(
                        out=AIT[:, sc * 128:(sc + 1) * 128], in_=tp
                    )
            # wfin, then AG1 staging LAST on the sync ring: ring FIFO puts the
            # staging transfer after every weight transfer, so the pending
            # collective can never starve the weight stream (v3 lost 17us to
            # exactly that; v4 lost 24us to the transposes queued behind it).
            nc.sync.dma_start(out=wfin[:, 0:4], in_=wfin_d[:, 0:4, :])
            nc.sync.dma_start(out=wfin[:, 4:8], in_=wfin_d[:, 4:8, :])
            nc.sync.dma_start(
                out=ag1_in[:].rearrange("(sc p) a -> p sc a", p=128), in_=AI_tok
            )
            nc.gpsimd.collective_compute(
                "AllGather", mybir.AluOpType.bypass, replica_groups=RG,
                ins=[ag1_in[:]], outs=[ag1_out[:]],
            )
            AIfull = acts.tile([128, TC_FULL, A], BF16)   # gathered AI token-major
            nc.gpsimd.dma_start(
                out=AIfull, in_=ag1_out[:].rearrange("(k p) a -> p k a", p=128)
            )

            # ---------- expert path, part 1 (injected into the up/gate loop) ---
            hsb = acts.tile([128, SC, E * A], BF16)   # h, token-major, e-outer
            mv8s = work.tile([128, SC, E, 2], F32, tag="mv8s", bufs=1)

            def expert_mm(sc):
                hp0 = ps_out.tile([128, 512], F32, tag="fout")
                hp1 = ps_out.tile([128, 512], F32, tag="fout")
                sl = AIT[:, sc * 128:(sc + 1) * 128]
                nc.tensor.matmul(hp0, sl, pk1[:, SC * A:SC * A + 512],
                                 start=True, stop=True)
                nc.tensor.matmul(hp1, sl, pk1[:, SC * A + 512:SC * A + 1024],
                                 start=True, stop=True)
                st8 = work.tile([128, E, 6], F32, tag="st8")
                hps = [hp0, hp0, hp0, hp0, hp1, hp1, hp1, hp1]
                for e in range(E):
                    nc.vector.bn_stats(
                        out=st8[:, e, :], in_=hps[e][:, (e % 4) * A:(e % 4 + 1) * A]
                    )
                for e in range(E):
                    nc.vector.bn_aggr(out=mv8s[:, sc, e, :], in_=st8[:, e, :])
                nc.scalar.copy(out=hsb[:, sc, 0:512], in_=hp0)
                nc.scalar.copy(out=hsb[:, sc, 512:1024], in_=hp1)

            def expert_ln(sc, r8all):
                rw8 = work.tile([128, E], F32, tag="rw8")
                nc.vector.tensor_tensor(
                    out=rw8, in0=r8all[:, sc, :],
                    in1=ewanc[:, sc * E:(sc + 1) * E],
                    op=mybir.AluOpType.mult,
                )
                nmrw = work.tile([128, E], F32, tag="nmrw")
                nc.vector.tensor_tensor(
                    out=nmrw, in0=mv8s[:, sc, :, 0], in1=rw8,
                    op=mybir.AluOpType.mult,
                )
                nc.vector.tensor_scalar(
                    out=nmrw, in0=nmrw, scalar1=-1.0, scalar2=None,
                    op0=mybir.AluOpType.mult,
                )
                zt = workbig.tile([128, E, A], BF16, tag="zt")
                for e in range(E):
                    nc.vector.tensor_scalar(
                        out=zt[:, e, :], in0=hsb[:, sc, e * A:(e + 1) * A],
                        scalar1=rw8[:, e:e + 1], scalar2=nmrw[:, e:e + 1],
                        op0=mybir.AluOpType.mult, op1=mybir.AluOpType.add,
                    )
                zg = workbig.tile([128, E, A], BF16, tag="ztg")
                nc.vector.tensor_tensor(
                    out=zg, in0=zt, in1=agB, op=mybir.AluOpType.mult
                )
                t1 = workbig.tile([128, 4, A], BF16, tag="sum1")
                nc.vector.tensor_tensor(
                    out=t1, in0=zg[:, 0:4, :], in1=zg[:, 4:8, :],
                    op=mybir.AluOpType.add,
                )
                t2 = work.tile([128, 2, A], BF16, tag="sum2")
                nc.vector.tensor_tensor(
                    out=t2, in0=t1[:, 0:2, :], in1=t1[:, 2:4, :],
                    op=mybir.AluOpType.add,
                )
                mx = work.tile([128, A], BF16, tag="mx")
                nc.vector.tensor_tensor(
                    out=mx, in0=t2[:, 0, :], in1=t2[:, 1, :], op=mybir.AluOpType.add
                )
                nc.vector.tensor_tensor(
                    out=mix_tok[:, sc, :], in0=mx,
                    in1=pk1[:, sc * A:(sc + 1) * A],
                    op=mybir.AluOpType.add,
                )

            # ---------- hidden = silu(x@gate.T) * (x@up.T), feature-major -------
            po_box = {}

            def post_step(k):
                if k == 0:
                    po_box["t"] = ps_po.tile([128, T], F32, tag="po", name="po_ps")
                nc.tensor.matmul(
                    po_box["t"], post_w[:, :, k], hid[:, k, :],
                    start=(k == 0), stop=(k == HC - 1),
                )

            for hc in range(HC):
                # hc=0 borrows the po/acc banks so the first pair doesn't WAR
                # on the pre-LN evacuations of the ps_big banks
                if hc == 0:
                    up_ps = ps_po.tile([128, T], F32, tag="po", name="up0")
                    gt_ps = ps_acc.tile([128, T], F32, tag="adps", name="gt0")
                else:
                    up_ps = ps_big.tile([128, T], F32, tag="mm")
                    gt_ps = ps_big.tile([128, T], F32, tag="mm")
                for k in range(KD):
                    nc.tensor.matmul(
                        up_ps, ug_w[:, hc, k, 0:128], xT[:, k, :],
                        start=(k == 0), stop=(k == KD - 1),
                    )
                for k in range(KD):
                    nc.tensor.matmul(
                        gt_ps, ug_w[:, hc, k, 128:256], xT[:, k, :],
                        start=(k == 0), stop=(k == KD - 1),
                    )
                sg = work2.tile([128, T], BF16, tag="sg")
                nc.scalar.activation(
                    out=sg, in_=gt_ps, func=mybir.ActivationFunctionType.Silu
                )
                nc.vector.tensor_tensor(
                    out=hid[:, hc, :], in0=sg, in1=up_ps, op=mybir.AluOpType.mult
                )
                if hc > 0:
                    post_step(hc - 1)
                if 3 <= hc <= 6:
                    expert_mm(hc - 3)
            post_step(HC - 1)

            # ---------- expert path, part 2 (runs mid-up/gate off mv8s) ------
            sd8all = work.tile([128, SC, E], F32, tag="sd8all", bufs=1)
            nc.scalar.activation(
                out=sd8all, in_=mv8s[:, :, :, 1],
                func=mybir.ActivationFunctionType.Sqrt, bias=eps_t, scale=1.0,
            )
            r8all = work.tile([128, SC, E], F32, tag="r8all", bufs=1)
            nc.vector.reciprocal(out=r8all, in_=sd8all)
            for sc in range(SC):
                expert_ln(sc, r8all)
            # mixed -> feature-major via PE transposes (ring-independent)
            for sc in range(SC):
                tp = ps_out.tile([128, 128], BF16, tag="fout")
                nc.tensor.transpose(tp, mix_tok[:, sc, :], ident[:])
                if sc % 2 == 0:
                    nc.scalar.copy(out=mixedT[:, sc * 128:(sc + 1) * 128], in_=tp)
                else:
                    nc.vector.tensor_copy(
                        out=mixedT[:, sc * 128:(sc + 1) * 128], in_=tp
                    )

            # ---------- adapt_out LN, feature-major ----------
            # stats via PE ones-matmuls; rstd via reciprocal_approx_fast on the
            # [1,T] row (1-lane DVE but only ~0.7us); broadcast back over
            # partitions with two K=1 ones-row matmuls into PSUM.
            AOf = acts.tile([128, T], BF16)
            nc.vector.tensor_copy(out=AOf, in_=po_box["t"])
            # dummy Sqrt pinned to the last hid chunk: forces the act-table
            # switch while the AOf evacuation runs on vector, so the real sqrt
            # below hits a warm table
            tdum = aoln.tile([1, 1], F32)
            nc.scalar.activation(
                out=tdum, in_=hid[0:1, 15, 0:1],
                func=mybir.ActivationFunctionType.Sqrt,
                bias=eps_t[0:1], scale=1.0,
            )
            sqf = aoln.tile([128, T], BF16)
            nc.vector.tensor_tensor(out=sqf, in0=AOf, in1=AOf, op=mybir.AluOpType.mult)
            s0 = ps_po.tile([1, T], F32, tag="po")
            nc.tensor.matmul(s0, ones_col, AOf, start=True, stop=True)
            s1 = ps_acc.tile([1, T], F32, tag="adps")
            nc.tensor.matmul(s1, ones_col, sqf, start=True, stop=True)
            # stats split across scalar (mean, mean^2) and vector (E[x^2],
            # var, rstd, mean*rstd); an_g is folded into the K=1 broadcast
            # matmuls (lhsT = raw an_g row), so the token-space transform is
            # two fused vector ops.
            mean_b = aoln.tile([1, T], F32)
            nc.scalar.activation(
                out=mean_b, in_=s0, func=mybir.ActivationFunctionType.Copy,
                scale=1.0 / A,
            )
            e2 = aoln.tile([1, T], F32)
            nc.vector.tensor_scalar(
                out=e2, in0=s1, scalar1=1.0 / A, scalar2=None,
                op0=mybir.AluOpType.mult,
            )
            m2 = aoln.tile([1, T], F32)
            nc.scalar.square(out=m2, in_=mean_b)
            varf = aoln.tile([1, T], F32)
            nc.vector.tensor_tensor(out=varf, in0=e2, in1=m2,
                                    op=mybir.AluOpType.subtract)
            sdf = aoln.tile([1, T], F32)
            nc.scalar.activation(
                out=sdf, in_=varf, func=mybir.ActivationFunctionType.Sqrt,
                bias=eps_t[0:1], scale=1.0,
            )
            r32 = aoln.tile([1, T], F32)
            nc.vector.reciprocal_approx_fast(out=r32, in_=sdf)
            mr32 = aoln.tile([1, T], F32)   # mean * rstd (subtracted later)
            nc.vector.tensor_tensor(out=mr32, in0=mean_b, in1=r32,
                                    op=mybir.AluOpType.mult)
            # halves of fd(0)/fd(1) fill the PE while the rstd row chain runs
            fd0_ps = ps_out.tile([128, T], F32, tag="fout", name="fd0_ps")
            fd1_ps = ps_out.tile([128, T], F32, tag="fout", name="fd1_ps")
            for k in range(8):
                nc.tensor.matmul(
                    fd0_ps, wfin[:, 0, k, :], hid[:, k, :],
                    start=(k == 0), stop=False,
                )
            for k in range(8):
                nc.tensor.matmul(
                    fd1_ps, wfin[:, 1, k, :], hid[:, k, :],
                    start=(k == 0), stop=False,
                )
            grB_ps = ps_big.tile([128, T], F32, tag="mm", name="grB_ps")
            nc.tensor.matmul(grB_ps, angb_sb[0:1, :], r32, start=True, stop=True)
            gmrB_ps = ps_big.tile([128, T], F32, tag="mm", name="gmrB_ps")
            nc.tensor.matmul(gmrB_ps, angb_sb[0:1, :], mr32, start=True, stop=True)
            z1 = aoln.tile([128, T], BF16)
            nc.vector.tensor_tensor(out=z1, in0=AOf, in1=grB_ps,
                                    op=mybir.AluOpType.mult)
            nc.vector.scalar_tensor_tensor(
                out=AOT, in0=z1, scalar=ewanc[:, SC * E + 1:SC * E + 2],
                in1=gmrB_ps, op0=mybir.AluOpType.add, op1=mybir.AluOpType.subtract,
            )
            # stage + trigger AG2
            nc.scalar.dma_start(out=ag2_in[:], in_=AOT)
            nc.gpsimd.collective_compute(
                "AllGather", mybir.AluOpType.bypass, replica_groups=RG,
                ins=[ag2_in[:]], outs=[ag2_out[:]],
            )
            # dummy Silu pinned to AOT: warm the Silu table for the bmm1 silus
            tdum2 = aoln.tile([1, 1], BF16)
            nc.scalar.activation(
                out=tdum2, in_=AOT[0:1, 0:1], func=mybir.ActivationFunctionType.Silu,
            )
            for c in range(GROUP):
                nc.gpsimd.dma_start(
                    out=AOTfull[:, c, :], in_=ag2_out[c * A:(c + 1) * A, :]
                )
            AOTf = AOTfull.rearrange("a c t -> a (c t)")

            # ---------- final_down (down contribution only; mixed+adapt land
            # in final_close so fd never waits on the expert path) ----------
            fd_open = {}

            def final_down(dc):
                if dc == 0:
                    op = fd0_ps
                    ks = range(8, HC)
                elif dc == 1:
                    op = fd1_ps
                    ks = range(8, HC)
                else:
                    op = ps_out.tile([128, T], F32, tag="fout")
                    ks = range(HC)
                for k in ks:
                    nc.tensor.matmul(
                        op, wfin[:, dc, k, :], hid[:, k, :],
                        start=(k == 0), stop=(k == HC - 1),
                    )
                if dc >= DC - 2:
                    # last two banks stay resident; final_close accumulates
                    # adapt+mixed on top and a scalar copy evacuates
                    fd_open[dc] = op
                elif dc % 2 == 0:
                    nc.scalar.copy(out=facc[:, dc, :], in_=op)
                else:
                    nc.vector.tensor_copy(out=facc[:, dc, :], in_=op)

            for dc in range(DC):
                final_down(dc)

            # ---------- w = silu(clip(AI_loc @ AO_full.T)); adapt = w.T chain ---
            ad_ps = ps_acc.tile([128, T], F32, tag="adps")
            wts_buf = {}

            def bmm1_step(j):
                w_ps = ps_big.tile([128, T], F32, tag="mm")
                nc.tensor.matmul(
                    w_ps, AOTf[:, j * 128:(j + 1) * 128], AIT, start=True, stop=True
                )
                wc = work2.tile([128, T], BF16, tag="wc")
                nc.vector.tensor_scalar(
                    out=wc, in0=w_ps, scalar1=-5.0, scalar2=5.0,
                    op0=mybir.AluOpType.max, op1=mybir.AluOpType.min,
                )
                wt = wtp.tile([128, T], BF16, tag="wts")
                nc.scalar.activation(
                    out=wt, in_=wc, func=mybir.ActivationFunctionType.Silu
                )
                wts_buf[j] = wt

            # depth-2 software pipeline: bmm2(j) trails bmm1(j+2) so the PE
            # never waits on the clip/silu stages
            for j in range(TC_FULL):
                bmm1_step(j)
                if j >= 2:
                    nc.tensor.matmul(
                        ad_ps, AIfull[:, j - 2, :], wts_buf.pop(j - 2),
                        start=(j - 2 == 0), stop=False,
                    )
            for j in (TC_FULL - 2, TC_FULL - 1):
                nc.tensor.matmul(
                    ad_ps, AIfull[:, j, :], wts_buf.pop(j),
                    start=False, stop=(j == TC_FULL - 1),
                )
            nc.scalar.copy(out=adaptT, in_=ad_ps)

            # ---------- finish output ----------
            def final_close(dc):
                if dc in fd_open:
                    op = fd_open[dc]
                    nc.tensor.matmul(
                        op, wfin[:, dc, HC, :], adaptT,
                        start=False, stop=False, skip_group_check=True,
                    )
                    nc.tensor.matmul(
                        op, wfin[:, dc, HC + 1, :], mixedT,
                        start=False, stop=True, skip_group_check=True,
                    )
                    ob = evac.tile([128, T], BF16, tag="ob")
                    nc.scalar.copy(out=ob, in_=op)
                else:
                    op = ps_big.tile([128, T], F32, tag="mm")
                    nc.tensor.matmul(
                        op, wfin[:, dc, HC, :], adaptT, start=True, stop=False,
                    )
                    nc.tensor.matmul(
                        op, wfin[:, dc, HC + 1, :], mixedT, start=False, stop=True,
                    )
                    ob = evac.tile([128, T], BF16, tag="ob")
                    nc.vector.tensor_tensor(
                        out=ob, in0=facc[:, dc, :], in1=op, op=mybir.AluOpType.add
                    )
                nc.sync.dma_start(out=out_d[dc * 128:(dc + 1) * 128, :], in_=ob)

            for dc in range(DC):
                final_close(dc)

    nc.compile()
    return nc


def kernel(
    x, expert_weights, up_w, gate_w, down_w, pre_w, post_w, an_g, an_b,
    adapt_proj_w, adapter_w, adapter_g, adapter_b, expert_proj_w, output_proj_w,
):
    x = np.asarray(x, np.float32)
    expert_weights = np.asarray(expert_weights, np.float32)
    bf = ml_dtypes.bfloat16

    if "nc" not in _CACHE:
        _CACHE["nc"] = _build()
    nc = _CACHE["nc"]

    def pack(w, kc):
        # [kc*128, F] -> [128, kc, F] (partition-major SBUF layout)
        f = w.shape[1]
        return np.ascontiguousarray(
            w.reshape(kc, 128, f).transpose(1, 0, 2)
        ).astype(bf)

    ug_wT = np.concatenate(
        [np.asarray(up_w, np.float32), np.asarray(gate_w, np.float32)], axis=0
    ).T                                                        # [D, 2H]
    # hc-major pack: [128, HC, KD*256]; per hc: KD chunks of (up 128 | gate 128)
    up_part = ug_wT[:, :H].reshape(KD, 128, HC, 128)
    gt_part = ug_wT[:, H:].reshape(KD, 128, HC, 128)
    ug_hc = np.ascontiguousarray(
        np.stack([up_part, gt_part], axis=3)        # [k, p, hc, 2, 128]
        .transpose(1, 2, 0, 3, 4)                   # [p, hc, k, 2, 128]
        .reshape(128, HC, KD * 256)
    ).astype(bf)
    pre_wT = np.asarray(pre_w, np.float32).T                   # [D, A]
    post_pack = np.ascontiguousarray(
        np.asarray(post_w, np.float32).T.reshape(HC, 128, A).transpose(1, 2, 0)
    ).astype(bf)                                               # [128, A, HC]
    adapter_wT = (
        np.asarray(adapter_w, np.float32).transpose(2, 0, 1).reshape(A, E * A)
    ).astype(bf)                                               # [A, E*A] (e-major)
    down_w = np.asarray(down_w, np.float32)
    w_da = 0.1 * (down_w @ np.asarray(adapt_proj_w, np.float32))       # [D, A]
    w_mo = np.asarray(output_proj_w, np.float32) @ np.asarray(
        expert_proj_w, np.float32
    )                                                                   # [D, A]
    wfin = np.concatenate([down_w.T, w_da.T, w_mo.T], axis=0)  # [2304, D]
    angb = np.stack(
        [np.asarray(an_g, np.float32), np.asarray(an_b, np.float32)], axis=0
    )                                                                   # [2, A]
    ancol = np.ascontiguousarray(angb.T)                                # [A, 2]
    ag_row = np.asarray(adapter_g, np.float32).reshape(1, A * E).astype(bf)  # e-major
    bias_mix = (expert_weights @ np.asarray(adapter_b, np.float32)).astype(bf)

    xf = x.reshape(N, D)
    shared = {
        "ug_hc": ug_hc, "pre_wT": pack(pre_wT, KD),
        "post_wT": post_pack,
        "wfin": np.ascontiguousarray(
            wfin.reshape(HC + 2, 128, DC, 128).transpose(1, 2, 0, 3)
            .reshape(128, DC, (HC + 2) * 128)
        ).astype(bf), "angb": angb,
        "ag_row": ag_row,
    }
    in_maps = []
    for c in range(NCORES):
        sl = slice(c * T, (c + 1) * T)
        ewc = np.ascontiguousarray(expert_weights[sl]).reshape(SC, 128, E)
        ew_c = np.ascontiguousarray(ewc.transpose(1, 0, 2)).reshape(128, SC * E)
        bmc = np.ascontiguousarray(bias_mix[sl]).reshape(SC, 128, A)
        bm_c = np.ascontiguousarray(bmc.transpose(1, 0, 2)).reshape(128, SC * A)
        pk1_c = np.concatenate(
            [bm_c, np.broadcast_to(adapter_wT, (128, E * A)).astype(bf)], axis=1
        )
        ewanc_c = np.concatenate(
            [ew_c.astype(np.float32),
             np.broadcast_to(ancol.reshape(A, 2), (128, 2)).astype(np.float32)],
            axis=1,
        )
        in_maps.append(
            dict(
                shared,
                xT=pack(np.ascontiguousarray(xf[sl].T), KD),
                pk1=np.ascontiguousarray(pk1_c),
                ewanc=np.ascontiguousarray(ewanc_c),
            )
        )

    try:
        res = run_bass_kernel_spmd(nc, in_maps, list(range(NCORES))).results
    except Exception:
        # axon workers occasionally hang up; one retry on a fresh dispatch
        import time

        time.sleep(10)
        res = run_bass_kernel_spmd(nc, in_maps, list(range(NCORES))).results
    out = np.empty((N, D), np.float32)
    for c in range(NCORES):
        out[c * T:(c + 1) * T] = np.asarray(res[c]["out"], np.float32).T
    return out.reshape(B, S, D)
